# revision 3
# baseline (speedup 1.0000x reference)
"""AttentionDecoderCell (attention + GRUCell + linear head) on 8 trn2 cores.

Sharding: pure data parallel over batch B=2048 -> 8 cores x 256 rows.
Weights (attention / GRU / output linears) are replicated to every core.

Per-core kernel (B_loc=256 = 2 partition-tiles of 128):
  logits  = [prev_h | y | 1] @ [attn_W^T ; attn_b]        (PE, transposed on chip)
  attn_e  = exp(logits - rowmax), sumexp via ACT accum    (softmax, unnormalized)
  ctx     = (sum_s diag(attn_e[:, s]).T @ enc[:, s, :]) * 1/sumexp
            -- 512 accumulating PE matmuls per tile; diagonals built 16-at-a-
               time on DVE from a replicated-identity constant (broadcast AP)
  GRU     = torch GRUCell semantics, gates r/z fused in one [128,512] psum
  out     = h_new @ out_W^T + out_b  (single DVE tensor_tensor_reduce)

encoder_output ([B,S,H] f32, 1 GiB total) is the memory roofline; it streams
batch-major so every DMA moves 2 MiB with 16 KiB contiguous per partition.
"""

import os
import sys

import numpy as np

try:
    import concourse.bass as bass  # noqa: F401
except ImportError:  # pragma: no cover
    for _p in ("/opt/trn_rl_repo", os.path.expanduser("~/.axon_site/_ro/trn_rl_repo")):
        if os.path.isdir(_p) and _p not in sys.path:
            sys.path.insert(0, _p)
    import concourse.bass as bass  # noqa: F401

import concourse.bacc as bacc
import concourse.tile as tile
from concourse import mybir
from concourse.masks import make_identity

f32 = mybir.dt.float32

B, S, H, F = 2048, 512, 256, 64
N_CORES = 8
B_LOC = B // N_CORES          # 256 rows per core
P = 128                       # partition tile
G = 16                        # s-timesteps per diag build / enc DMA chunk
ENC_BUFS = 5                  # in-flight 2 MiB enc chunks


def _bcast_free(ap_obj, n):
    """Append a stride-0 free dim of size n to an AP (free-dim broadcast)."""
    return bass.AP(
        tensor=ap_obj.tensor,
        offset=ap_obj.offset,
        ap=list(ap_obj.ap) + [[0, n]],
    )


def _bcast_part(dram_tensor, p, free_elems):
    """Partition-broadcast a DRAM row across p partitions (stride-0)."""
    return bass.AP(tensor=dram_tensor, offset=0, ap=[[0, p], [1, free_elems]])


def build_nc():
    nc = bacc.Bacc(None, target_bir_lowering=False, debug=False)

    enc = nc.dram_tensor("encoder_output", [B_LOC, S, H], f32, kind="ExternalInput")
    prev_h = nc.dram_tensor("prev_hidden", [B_LOC, H], f32, kind="ExternalInput")
    y_in = nc.dram_tensor("y", [B_LOC, F], f32, kind="ExternalInput")
    attn_W = nc.dram_tensor("attn_W", [S, H + F], f32, kind="ExternalInput")
    attn_b = nc.dram_tensor("attn_b", [S], f32, kind="ExternalInput")
    W_ih = nc.dram_tensor("W_ih", [3 * H, H], f32, kind="ExternalInput")
    W_hh = nc.dram_tensor("W_hh", [3 * H, H], f32, kind="ExternalInput")
    b_ih = nc.dram_tensor("b_ih", [3 * H], f32, kind="ExternalInput")
    b_hh = nc.dram_tensor("b_hh", [3 * H], f32, kind="ExternalInput")
    out_W = nc.dram_tensor("out_W", [1, H], f32, kind="ExternalInput")
    out_b = nc.dram_tensor("out_b", [1], f32, kind="ExternalInput")
    out_o = nc.dram_tensor("out", [B_LOC, 1], f32, kind="ExternalOutput")
    hnew_o = nc.dram_tensor("h_new", [B_LOC, H], f32, kind="ExternalOutput")

    with tile.TileContext(nc) as tc:
        with (
            tc.tile_pool(name="consts", bufs=1) as consts,
            tc.tile_pool(name="wstage", bufs=2) as wstage,
            tc.tile_pool(name="work", bufs=2) as work,
            tc.tile_pool(name="encp", bufs=ENC_BUFS) as encp,
            tc.tile_pool(name="diagp", bufs=3) as diagp,
            tc.tile_pool(name="ps_big", bufs=2, space="PSUM") as ps_big,
            tc.tile_pool(name="ps_ctx", bufs=2, space="PSUM") as ps_ctxp,
            tc.tile_pool(name="ps_n", bufs=2, space="PSUM") as ps_np,
            tc.tile_pool(name="ps_tr", bufs=2, space="PSUM") as ps_tr,
        ):
            # ---------------- constants / weight preprocessing ----------------
            ident = consts.tile([P, P], f32)
            make_identity(nc, ident[:])
            identG = consts.tile([P, G, P], f32)
            nc.gpsimd.memset(identG[:], 0.0)
            nc.gpsimd.affine_select(
                out=identG[:], in_=identG[:],
                compare_op=mybir.AluOpType.not_equal, fill=1.0, base=0,
                pattern=[[0, G], [-1, P]], channel_multiplier=1,
            )
            ones_row = consts.tile([1, P], f32)
            nc.vector.memset(ones_row[:], 1.0)

            # attn_W^T as three k-chunk tiles (k = attn_in index; chunk2 also
            # carries attn_b in row 64 so the ones-column adds the bias)
            awt0 = consts.tile([P, S], f32)
            awt1 = consts.tile([P, S], f32)
            awt2 = consts.tile([P, S], f32)   # rows 0:64 = y part, row 64 = attn_b
            for i in range(S // P):
                aw = wstage.tile([P, H + F], f32, tag="aw")
                nc.sync.dma_start(out=aw[:], in_=attn_W[i * P:(i + 1) * P, :])
                for j, (dst, kw) in enumerate([(awt0, P), (awt1, P), (awt2, F)]):
                    pt = ps_tr.tile([P, P], f32)
                    nc.tensor.transpose(
                        pt[0:kw, :], aw[:, j * P:j * P + kw], ident[:]
                    )
                    nc.vector.tensor_copy(
                        dst[0:kw, i * P:(i + 1) * P], pt[0:kw, :]
                    )
            nc.sync.dma_start(
                out=awt2[F:F + 1, :], in_=attn_b.rearrange("(a s) -> a s", a=1)
            )

            # W_ih^T / W_hh^T as [128, 768] k-chunk tiles
            wit = [consts.tile([P, 3 * H], f32, name=f"wit{j}") for j in range(2)]
            wht = [consts.tile([P, 3 * H], f32, name=f"wht{j}") for j in range(2)]
            for src, dsts in ((W_ih, wit), (W_hh, wht)):
                for i in range(3 * H // P):
                    wrow = wstage.tile([P, H], f32, tag="wrow")
                    nc.sync.dma_start(out=wrow[:], in_=src[i * P:(i + 1) * P, :])
                    for j in range(2):
                        pt = ps_tr.tile([P, P], f32)
                        nc.tensor.transpose(pt[:], wrow[:, j * P:(j + 1) * P], ident[:])
                        nc.vector.tensor_copy(dsts[j][:, i * P:(i + 1) * P], pt[:])
            bias_i = consts.tile([1, 3 * H], f32)
            bias_h = consts.tile([1, 3 * H], f32)
            nc.sync.dma_start(out=bias_i[:], in_=b_ih.rearrange("(a g) -> a g", a=1))
            nc.sync.dma_start(out=bias_h[:], in_=b_hh.rearrange("(a g) -> a g", a=1))

            outw_rep = consts.tile([P, H], f32)
            nc.gpsimd.dma_start(out=outw_rep[:], in_=_bcast_part(out_W, P, H))
            outb_col = consts.tile([P, 1], f32)
            nc.gpsimd.dma_start(out=outb_col[:], in_=_bcast_part(out_b, P, 1))

            # ---------------- per batch-tile pipeline ----------------
            for t in range(B_LOC // P):
                b0 = t * P
                h_tile = work.tile([P, H], f32, tag="h")
                nc.sync.dma_start(out=h_tile[:], in_=prev_h[b0:b0 + P, :])
                y_tile = work.tile([P, F], f32, tag="y")
                nc.sync.dma_start(out=y_tile[:], in_=y_in[b0:b0 + P, :])

                hT = work.tile([P, H], f32, tag="hT")
                for j in range(2):
                    pt = ps_tr.tile([P, P], f32)
                    nc.tensor.transpose(pt[:], h_tile[:, j * P:(j + 1) * P], ident[:])
                    nc.vector.tensor_copy(hT[:, j * P:(j + 1) * P], pt[:])
                yT1 = work.tile([F + 1, P], f32, tag="yT1")
                pt = ps_tr.tile([P, P], f32)
                nc.tensor.transpose(pt[0:F, :], y_tile[:], ident[:])
                nc.vector.tensor_copy(yT1[0:F, :], pt[0:F, :])
                nc.vector.memset(yT1[F:F + 1, :], 1.0)

                logits = ps_big.tile([P, S], f32, tag="big")
                nc.tensor.matmul(logits[:], hT[:, 0:P], awt0[:], start=True, stop=False)
                nc.tensor.matmul(logits[:], hT[:, P:H], awt1[:], start=False, stop=False)
                nc.tensor.matmul(
                    logits[:], yT1[0:F + 1, :], awt2[0:F + 1, :],
                    start=False, stop=True,
                )

                negmax = work.tile([P, 1], f32, tag="negmax")
                nc.vector.reduce_max(
                    negmax[:], logits[:], axis=mybir.AxisListType.X, negate=True
                )
                attn_e = work.tile([P, S], f32, tag="attn_e")
                sumexp = work.tile([P, 1], f32, tag="sumexp")
                nc.scalar.activation(
                    out=attn_e[:], in_=logits[:],
                    func=mybir.ActivationFunctionType.Exp,
                    bias=negmax[:], scale=1.0, accum_out=sumexp[:],
                )
                inv_sum = work.tile([P, 1], f32, tag="inv_sum")
                nc.vector.reciprocal(inv_sum[:], sumexp[:])

                # ctx accumulation: 512 diag matmuls, enc streamed in 2 MiB chunks
                ps_ctx = ps_ctxp.tile([P, H], f32, tag="ctx")
                for c in range(S // G):
                    enc_t = encp.tile([P, G, H], f32, tag="enc")
                    nc.sync.dma_start(
                        out=enc_t[:], in_=enc[b0:b0 + P, c * G:(c + 1) * G, :]
                    )
                    diag = diagp.tile([P, G, P], f32, tag="diag")
                    nc.vector.tensor_tensor(
                        out=diag[:], in0=identG[:],
                        in1=_bcast_free(attn_e[:, c * G:(c + 1) * G], P),
                        op=mybir.AluOpType.mult,
                    )
                    for j in range(G):
                        s = c * G + j
                        nc.tensor.matmul(
                            ps_ctx[:], lhsT=diag[:, j, :], rhs=enc_t[:, j, :],
                            start=(s == 0), stop=(s == S - 1),
                        )

                ctx_sb = work.tile([P, H], f32, tag="ctx_sb")
                nc.vector.tensor_scalar_mul(ctx_sb[:], ps_ctx[:], inv_sum[:])

                ctxT = work.tile([P, H], f32, tag="ctxT")
                for j in range(2):
                    pt = ps_tr.tile([P, P], f32)
                    nc.tensor.transpose(pt[:], ctx_sb[:, j * P:(j + 1) * P], ident[:])
                    nc.vector.tensor_copy(ctxT[:, j * P:(j + 1) * P], pt[:])

                # gates r,z fused: sigmoid(ctx@Wi[rz] + h@Wh[rz] + bi[rz] + bh[rz])
                RZ = 2 * H
                ps_rz = ps_big.tile([P, RZ], f32, tag="big")
                nc.tensor.matmul(ps_rz[:], ctxT[:, 0:P], wit[0][:, 0:RZ], start=True, stop=False)
                nc.tensor.matmul(ps_rz[:], ctxT[:, P:H], wit[1][:, 0:RZ], start=False, stop=False)
                nc.tensor.matmul(ps_rz[:], hT[:, 0:P], wht[0][:, 0:RZ], start=False, stop=False)
                nc.tensor.matmul(ps_rz[:], hT[:, P:H], wht[1][:, 0:RZ], start=False, stop=False)
                nc.tensor.matmul(ps_rz[:], ones_row[:], bias_i[0:1, 0:RZ], start=False, stop=False)
                nc.tensor.matmul(ps_rz[:], ones_row[:], bias_h[0:1, 0:RZ], start=False, stop=True)

                ps_ni = ps_np.tile([P, H], f32, tag="n")
                nc.tensor.matmul(ps_ni[:], ctxT[:, 0:P], wit[0][:, RZ:3 * H], start=True, stop=False)
                nc.tensor.matmul(ps_ni[:], ctxT[:, P:H], wit[1][:, RZ:3 * H], start=False, stop=False)
                nc.tensor.matmul(ps_ni[:], ones_row[:], bias_i[0:1, RZ:3 * H], start=False, stop=True)
                ps_nh = ps_np.tile([P, H], f32, tag="n")
                nc.tensor.matmul(ps_nh[:], hT[:, 0:P], wht[0][:, RZ:3 * H], start=True, stop=False)
                nc.tensor.matmul(ps_nh[:], hT[:, P:H], wht[1][:, RZ:3 * H], start=False, stop=False)
                nc.tensor.matmul(ps_nh[:], ones_row[:], bias_h[0:1, RZ:3 * H], start=False, stop=True)

                rz_sb = work.tile([P, RZ], f32, tag="rz")
                nc.scalar.activation(
                    out=rz_sb[:], in_=ps_rz[:],
                    func=mybir.ActivationFunctionType.Sigmoid,
                )
                # n = tanh(i_n + r * h_n)
                rhn = work.tile([P, H], f32, tag="rhn")
                nc.vector.tensor_tensor(
                    out=rhn[:], in0=rz_sb[:, 0:H], in1=ps_nh[:],
                    op=mybir.AluOpType.mult,
                )
                pre_n = work.tile([P, H], f32, tag="pre_n")
                nc.vector.tensor_tensor(
                    out=pre_n[:], in0=rhn[:], in1=ps_ni[:], op=mybir.AluOpType.add
                )
                n_sb = work.tile([P, H], f32, tag="n_sb")
                nc.scalar.activation(
                    out=n_sb[:], in_=pre_n[:],
                    func=mybir.ActivationFunctionType.Tanh,
                )
                # h_new = n + z * (h - n)
                d_sb = work.tile([P, H], f32, tag="d_sb")
                nc.vector.tensor_tensor(
                    out=d_sb[:], in0=h_tile[:], in1=n_sb[:],
                    op=mybir.AluOpType.subtract,
                )
                zd = work.tile([P, H], f32, tag="zd")
                nc.vector.tensor_tensor(
                    out=zd[:], in0=rz_sb[:, H:RZ], in1=d_sb[:],
                    op=mybir.AluOpType.mult,
                )
                hnew_sb = work.tile([P, H], f32, tag="hnew")
                nc.vector.tensor_tensor(
                    out=hnew_sb[:], in0=zd[:], in1=n_sb[:], op=mybir.AluOpType.add
                )
                nc.sync.dma_start(out=hnew_o[b0:b0 + P, :], in_=hnew_sb[:])

                # out = h_new @ out_W^T + out_b
                # (tensor_tensor_reduce would fuse this, but its lowering
                # wedges the device on this runtime build — keep it split)
                prod = work.tile([P, H], f32, tag="prod")
                out_col = work.tile([P, 1], f32, tag="out_col")
                nc.vector.tensor_tensor(
                    out=prod[:], in0=hnew_sb[:], in1=outw_rep[:],
                    op=mybir.AluOpType.mult,
                )
                nc.vector.reduce_sum(
                    out_col[:], prod[:], axis=mybir.AxisListType.X
                )
                nc.vector.tensor_scalar_add(out_col[:], out_col[:], outb_col[:])
                nc.sync.dma_start(out=out_o[b0:b0 + P, :], in_=out_col[:])

    nc.compile()
    return nc


_NC_CACHE = None


def _get_nc():
    global _NC_CACHE
    if _NC_CACHE is None:
        _NC_CACHE = build_nc()
    return _NC_CACHE


def kernel(encoder_output, prev_hidden, y, attn_W, attn_b, W_ih, W_hh,
           b_ih, b_hh, out_W, out_b, **run_kwargs):
    from concourse.bass_utils import run_bass_kernel_spmd

    encoder_output = np.ascontiguousarray(np.asarray(encoder_output, np.float32))
    prev_hidden = np.ascontiguousarray(np.asarray(prev_hidden, np.float32))
    y = np.ascontiguousarray(np.asarray(y, np.float32))
    shared = {
        "attn_W": np.ascontiguousarray(np.asarray(attn_W, np.float32)),
        "attn_b": np.ascontiguousarray(np.asarray(attn_b, np.float32)),
        "W_ih": np.ascontiguousarray(np.asarray(W_ih, np.float32)),
        "W_hh": np.ascontiguousarray(np.asarray(W_hh, np.float32)),
        "b_ih": np.ascontiguousarray(np.asarray(b_ih, np.float32)),
        "b_hh": np.ascontiguousarray(np.asarray(b_hh, np.float32)),
        "out_W": np.ascontiguousarray(np.asarray(out_W, np.float32)),
        "out_b": np.ascontiguousarray(np.asarray(out_b, np.float32)),
    }
    in_maps = []
    for i in range(N_CORES):
        lo, hi = i * B_LOC, (i + 1) * B_LOC
        in_maps.append({
            "encoder_output": encoder_output[lo:hi],
            "prev_hidden": prev_hidden[lo:hi],
            "y": y[lo:hi],
            **shared,
        })

    nc = _get_nc()
    res = run_bass_kernel_spmd(nc, in_maps, core_ids=list(range(N_CORES)),
                               **run_kwargs)
    out = np.concatenate([res.results[i]["out"] for i in range(N_CORES)], axis=0)
    h_new = np.concatenate([res.results[i]["h_new"] for i in range(N_CORES)], axis=0)
    kernel.last_results = res
    return (out, h_new)


# revision 4
# speedup vs baseline: 1.2016x; 1.2016x over previous
"""AttentionDecoderCell (attention + GRUCell + linear head) on 8 trn2 cores.

Sharding: pure data parallel over batch B=2048 -> 8 cores x 256 rows.
Weights (attention / GRU / output linears) are replicated to every core.

Per-core kernel (B_loc=256 = 2 partition-tiles of 128):
  logits  = [prev_h | y | 1] @ [attn_W^T ; attn_b]        (PE, transposed on chip)
  attn_e  = exp(logits - rowmax), sumexp via ACT accum    (softmax, unnormalized)
  ctx     = (sum_s diag(attn_e[:, s]).T @ enc[:, s, :]) * 1/sumexp
            -- 512 accumulating PE matmuls per tile; diagonals built 16-at-a-
               time on DVE from a replicated-identity constant (broadcast AP)
  GRU     = torch GRUCell semantics, gates r/z fused in one [128,512] psum
  out     = h_new @ out_W^T + out_b  (single DVE tensor_tensor_reduce)

encoder_output ([B,S,H] f32, 1 GiB total) is the memory roofline; it streams
batch-major so every DMA moves 2 MiB with 16 KiB contiguous per partition.
"""

import os
import sys

import numpy as np

try:
    import concourse.bass as bass  # noqa: F401
except ImportError:  # pragma: no cover
    for _p in ("/opt/trn_rl_repo", os.path.expanduser("~/.axon_site/_ro/trn_rl_repo")):
        if os.path.isdir(_p) and _p not in sys.path:
            sys.path.insert(0, _p)
    import concourse.bass as bass  # noqa: F401

import concourse.bacc as bacc
import concourse.tile as tile
from concourse import mybir
from concourse.masks import make_identity

f32 = mybir.dt.float32
bf16 = mybir.dt.bfloat16

B, S, H, F = 2048, 512, 256, 64
N_CORES = 8
B_LOC = B // N_CORES          # 256 rows per core
P = 128                       # partition tile
G = 16                        # s-timesteps per diag build / enc DMA chunk
ENC_BUFS = 8                  # in-flight 2 MiB enc chunks


def _bcast_free(ap_obj, n):
    """Append a stride-0 free dim of size n to an AP (free-dim broadcast)."""
    return bass.AP(
        tensor=ap_obj.tensor,
        offset=ap_obj.offset,
        ap=list(ap_obj.ap) + [[0, n]],
    )


def _bcast_part(dram_tensor, p, free_elems):
    """Partition-broadcast a DRAM row across p partitions (stride-0)."""
    return bass.AP(tensor=dram_tensor, offset=0, ap=[[0, p], [1, free_elems]])


def build_nc():
    nc = bacc.Bacc(None, target_bir_lowering=False, debug=False)

    enc = nc.dram_tensor("encoder_output", [B_LOC, S, H], f32, kind="ExternalInput")
    prev_h = nc.dram_tensor("prev_hidden", [B_LOC, H], f32, kind="ExternalInput")
    y_in = nc.dram_tensor("y", [B_LOC, F], f32, kind="ExternalInput")
    attn_W = nc.dram_tensor("attn_W", [S, H + F], f32, kind="ExternalInput")
    attn_b = nc.dram_tensor("attn_b", [S], f32, kind="ExternalInput")
    W_ih = nc.dram_tensor("W_ih", [3 * H, H], f32, kind="ExternalInput")
    W_hh = nc.dram_tensor("W_hh", [3 * H, H], f32, kind="ExternalInput")
    b_ih = nc.dram_tensor("b_ih", [3 * H], f32, kind="ExternalInput")
    b_hh = nc.dram_tensor("b_hh", [3 * H], f32, kind="ExternalInput")
    out_W = nc.dram_tensor("out_W", [1, H], f32, kind="ExternalInput")
    out_b = nc.dram_tensor("out_b", [1], f32, kind="ExternalInput")
    out_o = nc.dram_tensor("out", [B_LOC, 1], f32, kind="ExternalOutput")
    hnew_o = nc.dram_tensor("h_new", [B_LOC, H], f32, kind="ExternalOutput")

    with tile.TileContext(nc) as tc:
        with (
            tc.tile_pool(name="consts", bufs=1) as consts,
            tc.tile_pool(name="wstage", bufs=2) as wstage,
            tc.tile_pool(name="work", bufs=2) as work,
            tc.tile_pool(name="encp", bufs=ENC_BUFS) as encp,
            tc.tile_pool(name="diagp", bufs=3) as diagp,
            tc.tile_pool(name="ps_big", bufs=2, space="PSUM") as ps_big,
            tc.tile_pool(name="ps_ctx", bufs=2, space="PSUM") as ps_ctxp,
            tc.tile_pool(name="ps_n", bufs=2, space="PSUM") as ps_np,
            tc.tile_pool(name="ps_tr", bufs=2, space="PSUM") as ps_tr,
        ):
            # ---------------- constants / weight preprocessing ----------------
            ident = consts.tile([P, P], f32)
            make_identity(nc, ident[:])
            identG = consts.tile([P, G, P], f32)
            nc.gpsimd.memset(identG[:], 0.0)
            nc.gpsimd.affine_select(
                out=identG[:], in_=identG[:],
                compare_op=mybir.AluOpType.not_equal, fill=1.0, base=0,
                pattern=[[0, G], [-1, P]], channel_multiplier=1,
            )
            ones_row = consts.tile([1, P], f32)
            nc.vector.memset(ones_row[:], 1.0)

            # attn_W^T as three k-chunk tiles (k = attn_in index; chunk2 also
            # carries attn_b in row 64 so the ones-column adds the bias)
            awt0 = consts.tile([P, S], f32)
            awt1 = consts.tile([P, S], f32)
            awt2 = consts.tile([P, S], f32)   # rows 0:64 = y part, row 64 = attn_b
            for i in range(S // P):
                aw = wstage.tile([P, H + F], f32, tag="aw")
                nc.sync.dma_start(out=aw[:], in_=attn_W[i * P:(i + 1) * P, :])
                for j, (dst, kw) in enumerate([(awt0, P), (awt1, P), (awt2, F)]):
                    pt = ps_tr.tile([P, P], f32)
                    nc.tensor.transpose(
                        pt[0:kw, :], aw[:, j * P:j * P + kw], ident[:]
                    )
                    nc.vector.tensor_copy(
                        dst[0:kw, i * P:(i + 1) * P], pt[0:kw, :]
                    )
            nc.sync.dma_start(
                out=awt2[F:F + 1, :], in_=attn_b.rearrange("(a s) -> a s", a=1)
            )

            # W_ih^T / W_hh^T as [128, 768] k-chunk tiles
            wit = [consts.tile([P, 3 * H], f32, name=f"wit{j}") for j in range(2)]
            wht = [consts.tile([P, 3 * H], f32, name=f"wht{j}") for j in range(2)]
            for src, dsts in ((W_ih, wit), (W_hh, wht)):
                for i in range(3 * H // P):
                    wrow = wstage.tile([P, H], f32, tag="wrow")
                    nc.sync.dma_start(out=wrow[:], in_=src[i * P:(i + 1) * P, :])
                    for j in range(2):
                        pt = ps_tr.tile([P, P], f32)
                        nc.tensor.transpose(pt[:], wrow[:, j * P:(j + 1) * P], ident[:])
                        nc.vector.tensor_copy(dsts[j][:, i * P:(i + 1) * P], pt[:])
            bias_i = consts.tile([1, 3 * H], f32)
            bias_h = consts.tile([1, 3 * H], f32)
            nc.sync.dma_start(out=bias_i[:], in_=b_ih.rearrange("(a g) -> a g", a=1))
            nc.sync.dma_start(out=bias_h[:], in_=b_hh.rearrange("(a g) -> a g", a=1))

            outw_rep = consts.tile([P, H], f32)
            nc.gpsimd.dma_start(out=outw_rep[:], in_=_bcast_part(out_W, P, H))
            outb_col = consts.tile([P, 1], f32)
            nc.gpsimd.dma_start(out=outb_col[:], in_=_bcast_part(out_b, P, 1))

            # ---------------- per batch-tile pipeline ----------------
            for t in range(B_LOC // P):
                b0 = t * P
                h_tile = work.tile([P, H], f32, tag="h")
                nc.sync.dma_start(out=h_tile[:], in_=prev_h[b0:b0 + P, :])
                y_tile = work.tile([P, F], f32, tag="y")
                nc.sync.dma_start(out=y_tile[:], in_=y_in[b0:b0 + P, :])

                hT = work.tile([P, H], f32, tag="hT")
                for j in range(2):
                    pt = ps_tr.tile([P, P], f32)
                    nc.tensor.transpose(pt[:], h_tile[:, j * P:(j + 1) * P], ident[:])
                    nc.vector.tensor_copy(hT[:, j * P:(j + 1) * P], pt[:])
                yT1 = work.tile([F + 1, P], f32, tag="yT1")
                pt = ps_tr.tile([P, P], f32)
                nc.tensor.transpose(pt[0:F, :], y_tile[:], ident[:])
                nc.vector.tensor_copy(yT1[0:F, :], pt[0:F, :])
                nc.vector.memset(yT1[F:F + 1, :], 1.0)

                logits = ps_big.tile([P, S], f32, tag="big")
                nc.tensor.matmul(logits[:], hT[:, 0:P], awt0[:], start=True, stop=False)
                nc.tensor.matmul(logits[:], hT[:, P:H], awt1[:], start=False, stop=False)
                nc.tensor.matmul(
                    logits[:], yT1[0:F + 1, :], awt2[0:F + 1, :],
                    start=False, stop=True,
                )

                negmax = work.tile([P, 1], f32, tag="negmax")
                nc.vector.reduce_max(
                    negmax[:], logits[:], axis=mybir.AxisListType.X, negate=True
                )
                attn_e = work.tile([P, S], f32, tag="attn_e")
                sumexp = work.tile([P, 1], f32, tag="sumexp")
                nc.scalar.activation(
                    out=attn_e[:], in_=logits[:],
                    func=mybir.ActivationFunctionType.Exp,
                    bias=negmax[:], scale=1.0, accum_out=sumexp[:],
                )
                inv_sum = work.tile([P, 1], f32, tag="inv_sum")
                nc.vector.reciprocal(inv_sum[:], sumexp[:])

                # ctx accumulation: 512 diag matmuls, enc streamed in 2 MiB chunks
                ps_ctx = ps_ctxp.tile([P, H], f32, tag="ctx")
                for c in range(S // G):
                    enc_t = encp.tile([P, G, H], bf16, tag="enc")
                    # SWDGE casts fp32->bf16 inline; HBM reads stay fp32
                    nc.gpsimd.dma_start(
                        out=enc_t[:], in_=enc[b0:b0 + P, c * G:(c + 1) * G, :]
                    )
                    diag = diagp.tile([P, G, P], bf16, tag="diag")
                    nc.vector.tensor_tensor(
                        out=diag[:], in0=identG[:],
                        in1=_bcast_free(attn_e[:, c * G:(c + 1) * G], P),
                        op=mybir.AluOpType.mult,
                    )
                    for j in range(G):
                        s = c * G + j
                        nc.tensor.matmul(
                            ps_ctx[:], lhsT=diag[:, j, :], rhs=enc_t[:, j, :],
                            start=(s == 0), stop=(s == S - 1),
                        )

                ctx_sb = work.tile([P, H], f32, tag="ctx_sb")
                nc.vector.tensor_scalar_mul(ctx_sb[:], ps_ctx[:], inv_sum[:])

                ctxT = work.tile([P, H], f32, tag="ctxT")
                for j in range(2):
                    pt = ps_tr.tile([P, P], f32)
                    nc.tensor.transpose(pt[:], ctx_sb[:, j * P:(j + 1) * P], ident[:])
                    nc.vector.tensor_copy(ctxT[:, j * P:(j + 1) * P], pt[:])

                # gates r,z fused: sigmoid(ctx@Wi[rz] + h@Wh[rz] + bi[rz] + bh[rz])
                RZ = 2 * H
                ps_rz = ps_big.tile([P, RZ], f32, tag="big")
                nc.tensor.matmul(ps_rz[:], ctxT[:, 0:P], wit[0][:, 0:RZ], start=True, stop=False)
                nc.tensor.matmul(ps_rz[:], ctxT[:, P:H], wit[1][:, 0:RZ], start=False, stop=False)
                nc.tensor.matmul(ps_rz[:], hT[:, 0:P], wht[0][:, 0:RZ], start=False, stop=False)
                nc.tensor.matmul(ps_rz[:], hT[:, P:H], wht[1][:, 0:RZ], start=False, stop=False)
                nc.tensor.matmul(ps_rz[:], ones_row[:], bias_i[0:1, 0:RZ], start=False, stop=False)
                nc.tensor.matmul(ps_rz[:], ones_row[:], bias_h[0:1, 0:RZ], start=False, stop=True)

                ps_ni = ps_np.tile([P, H], f32, tag="n")
                nc.tensor.matmul(ps_ni[:], ctxT[:, 0:P], wit[0][:, RZ:3 * H], start=True, stop=False)
                nc.tensor.matmul(ps_ni[:], ctxT[:, P:H], wit[1][:, RZ:3 * H], start=False, stop=False)
                nc.tensor.matmul(ps_ni[:], ones_row[:], bias_i[0:1, RZ:3 * H], start=False, stop=True)
                ps_nh = ps_np.tile([P, H], f32, tag="n")
                nc.tensor.matmul(ps_nh[:], hT[:, 0:P], wht[0][:, RZ:3 * H], start=True, stop=False)
                nc.tensor.matmul(ps_nh[:], hT[:, P:H], wht[1][:, RZ:3 * H], start=False, stop=False)
                nc.tensor.matmul(ps_nh[:], ones_row[:], bias_h[0:1, RZ:3 * H], start=False, stop=True)

                rz_sb = work.tile([P, RZ], f32, tag="rz")
                nc.scalar.activation(
                    out=rz_sb[:], in_=ps_rz[:],
                    func=mybir.ActivationFunctionType.Sigmoid,
                )
                # n = tanh(i_n + r * h_n)
                rhn = work.tile([P, H], f32, tag="rhn")
                nc.vector.tensor_tensor(
                    out=rhn[:], in0=rz_sb[:, 0:H], in1=ps_nh[:],
                    op=mybir.AluOpType.mult,
                )
                pre_n = work.tile([P, H], f32, tag="pre_n")
                nc.vector.tensor_tensor(
                    out=pre_n[:], in0=rhn[:], in1=ps_ni[:], op=mybir.AluOpType.add
                )
                n_sb = work.tile([P, H], f32, tag="n_sb")
                nc.scalar.activation(
                    out=n_sb[:], in_=pre_n[:],
                    func=mybir.ActivationFunctionType.Tanh,
                )
                # h_new = n + z * (h - n)
                d_sb = work.tile([P, H], f32, tag="d_sb")
                nc.vector.tensor_tensor(
                    out=d_sb[:], in0=h_tile[:], in1=n_sb[:],
                    op=mybir.AluOpType.subtract,
                )
                zd = work.tile([P, H], f32, tag="zd")
                nc.vector.tensor_tensor(
                    out=zd[:], in0=rz_sb[:, H:RZ], in1=d_sb[:],
                    op=mybir.AluOpType.mult,
                )
                hnew_sb = work.tile([P, H], f32, tag="hnew")
                nc.vector.tensor_tensor(
                    out=hnew_sb[:], in0=zd[:], in1=n_sb[:], op=mybir.AluOpType.add
                )
                nc.sync.dma_start(out=hnew_o[b0:b0 + P, :], in_=hnew_sb[:])

                # out = h_new @ out_W^T + out_b
                # (tensor_tensor_reduce would fuse this, but its lowering
                # wedges the device on this runtime build — keep it split)
                prod = work.tile([P, H], f32, tag="prod")
                out_col = work.tile([P, 1], f32, tag="out_col")
                nc.vector.tensor_tensor(
                    out=prod[:], in0=hnew_sb[:], in1=outw_rep[:],
                    op=mybir.AluOpType.mult,
                )
                nc.vector.reduce_sum(
                    out_col[:], prod[:], axis=mybir.AxisListType.X
                )
                nc.vector.tensor_scalar_add(out_col[:], out_col[:], outb_col[:])
                nc.sync.dma_start(out=out_o[b0:b0 + P, :], in_=out_col[:])

    nc.compile()
    return nc


_NC_CACHE = None


def _get_nc():
    global _NC_CACHE
    if _NC_CACHE is None:
        _NC_CACHE = build_nc()
    return _NC_CACHE


def kernel(encoder_output, prev_hidden, y, attn_W, attn_b, W_ih, W_hh,
           b_ih, b_hh, out_W, out_b, **run_kwargs):
    from concourse.bass_utils import run_bass_kernel_spmd

    encoder_output = np.ascontiguousarray(np.asarray(encoder_output, np.float32))
    prev_hidden = np.ascontiguousarray(np.asarray(prev_hidden, np.float32))
    y = np.ascontiguousarray(np.asarray(y, np.float32))
    shared = {
        "attn_W": np.ascontiguousarray(np.asarray(attn_W, np.float32)),
        "attn_b": np.ascontiguousarray(np.asarray(attn_b, np.float32)),
        "W_ih": np.ascontiguousarray(np.asarray(W_ih, np.float32)),
        "W_hh": np.ascontiguousarray(np.asarray(W_hh, np.float32)),
        "b_ih": np.ascontiguousarray(np.asarray(b_ih, np.float32)),
        "b_hh": np.ascontiguousarray(np.asarray(b_hh, np.float32)),
        "out_W": np.ascontiguousarray(np.asarray(out_W, np.float32)),
        "out_b": np.ascontiguousarray(np.asarray(out_b, np.float32)),
    }
    in_maps = []
    for i in range(N_CORES):
        lo, hi = i * B_LOC, (i + 1) * B_LOC
        in_maps.append({
            "encoder_output": encoder_output[lo:hi],
            "prev_hidden": prev_hidden[lo:hi],
            "y": y[lo:hi],
            **shared,
        })

    nc = _get_nc()
    res = run_bass_kernel_spmd(nc, in_maps, core_ids=list(range(N_CORES)),
                               **run_kwargs)
    out = np.concatenate([res.results[i]["out"] for i in range(N_CORES)], axis=0)
    h_new = np.concatenate([res.results[i]["h_new"] for i in range(N_CORES)], axis=0)
    kernel.last_results = res
    return (out, h_new)


# revision 5
# speedup vs baseline: 1.3367x; 1.1125x over previous
"""AttentionDecoderCell (attention + GRUCell + linear head) on 8 trn2 cores.

Sharding: pure data parallel over batch B=2048 -> 8 cores x 256 rows.
Weights (attention / GRU / output linears) are replicated to every core.

Per-core kernel (B_loc=256 = 2 partition-tiles of 128):
  logits  = [prev_h | y | 1] @ [attn_W^T ; attn_b]        (PE, transposed on chip)
  attn_e  = exp(logits - rowmax), sumexp via ACT accum    (softmax, unnormalized)
  ctx     = (sum_s diag(attn_e[:, s]).T @ enc[:, s, :]) * 1/sumexp
            -- 512 accumulating PE matmuls per tile; diagonals built 16-at-a-
               time on DVE from a replicated-identity constant (broadcast AP)
  GRU     = torch GRUCell semantics, gates r/z fused in one [128,512] psum
  out     = h_new @ out_W^T + out_b  (single DVE tensor_tensor_reduce)

encoder_output ([B,S,H] f32, 1 GiB total) is the memory roofline; it streams
batch-major so every DMA moves 2 MiB with 16 KiB contiguous per partition.
"""

import os
import sys

import numpy as np

try:
    import concourse.bass as bass  # noqa: F401
except ImportError:  # pragma: no cover
    for _p in ("/opt/trn_rl_repo", os.path.expanduser("~/.axon_site/_ro/trn_rl_repo")):
        if os.path.isdir(_p) and _p not in sys.path:
            sys.path.insert(0, _p)
    import concourse.bass as bass  # noqa: F401

import concourse.bacc as bacc
import concourse.tile as tile
from concourse import mybir
from concourse.masks import make_identity

f32 = mybir.dt.float32
bf16 = mybir.dt.bfloat16

B, S, H, F = 2048, 512, 256, 64
N_CORES = 8
B_LOC = B // N_CORES          # 256 rows per core
P = 128                       # partition tile
G = 32                        # s-timesteps per diag build / enc DMA chunk
ENC_BUFS = 6                  # in-flight 4 MiB enc chunks


def _bcast_free(ap_obj, n):
    """Append a stride-0 free dim of size n to an AP (free-dim broadcast)."""
    return bass.AP(
        tensor=ap_obj.tensor,
        offset=ap_obj.offset,
        ap=list(ap_obj.ap) + [[0, n]],
    )


def _bcast_part(dram_tensor, p, free_elems):
    """Partition-broadcast a DRAM row across p partitions (stride-0)."""
    return bass.AP(tensor=dram_tensor, offset=0, ap=[[0, p], [1, free_elems]])


def build_nc():
    nc = bacc.Bacc(None, target_bir_lowering=False, debug=False)

    enc = nc.dram_tensor("encoder_output", [B_LOC, S, H], f32, kind="ExternalInput")
    prev_h = nc.dram_tensor("prev_hidden", [B_LOC, H], f32, kind="ExternalInput")
    y_in = nc.dram_tensor("y", [B_LOC, F], f32, kind="ExternalInput")
    attn_W = nc.dram_tensor("attn_W", [S, H + F], f32, kind="ExternalInput")
    attn_b = nc.dram_tensor("attn_b", [S], f32, kind="ExternalInput")
    W_ih = nc.dram_tensor("W_ih", [3 * H, H], f32, kind="ExternalInput")
    W_hh = nc.dram_tensor("W_hh", [3 * H, H], f32, kind="ExternalInput")
    b_ih = nc.dram_tensor("b_ih", [3 * H], f32, kind="ExternalInput")
    b_hh = nc.dram_tensor("b_hh", [3 * H], f32, kind="ExternalInput")
    out_W = nc.dram_tensor("out_W", [1, H], f32, kind="ExternalInput")
    out_b = nc.dram_tensor("out_b", [1], f32, kind="ExternalInput")
    out_o = nc.dram_tensor("out", [B_LOC, 1], f32, kind="ExternalOutput")
    hnew_o = nc.dram_tensor("h_new", [B_LOC, H], f32, kind="ExternalOutput")

    with tile.TileContext(nc) as tc:
        with (
            tc.tile_pool(name="consts", bufs=1) as consts,
            tc.tile_pool(name="wstage", bufs=2) as wstage,
            tc.tile_pool(name="work", bufs=2) as work,
            tc.tile_pool(name="encp", bufs=ENC_BUFS) as encp,
            tc.tile_pool(name="diagp", bufs=3) as diagp,
            tc.tile_pool(name="ps_big", bufs=2, space="PSUM") as ps_big,
            tc.tile_pool(name="ps_ctx", bufs=2, space="PSUM") as ps_ctxp,
            tc.tile_pool(name="ps_n", bufs=2, space="PSUM") as ps_np,
            tc.tile_pool(name="ps_tr", bufs=2, space="PSUM") as ps_tr,
        ):
            # ---------------- constants / weight preprocessing ----------------
            ident = consts.tile([P, P], f32)
            make_identity(nc, ident[:])
            identG = consts.tile([P, G, P], f32)
            nc.gpsimd.memset(identG[:], 0.0)
            nc.gpsimd.affine_select(
                out=identG[:], in_=identG[:],
                compare_op=mybir.AluOpType.not_equal, fill=1.0, base=0,
                pattern=[[0, G], [-1, P]], channel_multiplier=1,
            )
            ones_row = consts.tile([1, P], f32)
            nc.vector.memset(ones_row[:], 1.0)

            # attn_W^T as three k-chunk tiles (k = attn_in index; chunk2 also
            # carries attn_b in row 64 so the ones-column adds the bias)
            awt0 = consts.tile([P, S], f32)
            awt1 = consts.tile([P, S], f32)
            awt2 = consts.tile([P, S], f32)   # rows 0:64 = y part, row 64 = attn_b
            for i in range(S // P):
                aw = wstage.tile([P, H + F], f32, tag="aw")
                nc.sync.dma_start(out=aw[:], in_=attn_W[i * P:(i + 1) * P, :])
                for j, (dst, kw) in enumerate([(awt0, P), (awt1, P), (awt2, F)]):
                    pt = ps_tr.tile([P, P], f32)
                    nc.tensor.transpose(
                        pt[0:kw, :], aw[:, j * P:j * P + kw], ident[:]
                    )
                    nc.vector.tensor_copy(
                        dst[0:kw, i * P:(i + 1) * P], pt[0:kw, :]
                    )
            nc.sync.dma_start(
                out=awt2[F:F + 1, :], in_=attn_b.rearrange("(a s) -> a s", a=1)
            )

            # W_ih^T / W_hh^T as [128, 768] k-chunk tiles
            wit = [consts.tile([P, 3 * H], f32, name=f"wit{j}") for j in range(2)]
            wht = [consts.tile([P, 3 * H], f32, name=f"wht{j}") for j in range(2)]
            for src, dsts in ((W_ih, wit), (W_hh, wht)):
                for i in range(3 * H // P):
                    wrow = wstage.tile([P, H], f32, tag="wrow")
                    nc.sync.dma_start(out=wrow[:], in_=src[i * P:(i + 1) * P, :])
                    for j in range(2):
                        pt = ps_tr.tile([P, P], f32)
                        nc.tensor.transpose(pt[:], wrow[:, j * P:(j + 1) * P], ident[:])
                        nc.vector.tensor_copy(dsts[j][:, i * P:(i + 1) * P], pt[:])
            bias_i = consts.tile([1, 3 * H], f32)
            bias_h = consts.tile([1, 3 * H], f32)
            nc.sync.dma_start(out=bias_i[:], in_=b_ih.rearrange("(a g) -> a g", a=1))
            nc.sync.dma_start(out=bias_h[:], in_=b_hh.rearrange("(a g) -> a g", a=1))

            outw_rep = consts.tile([P, H], f32)
            nc.gpsimd.dma_start(out=outw_rep[:], in_=_bcast_part(out_W, P, H))
            outb_col = consts.tile([P, 1], f32)
            nc.gpsimd.dma_start(out=outb_col[:], in_=_bcast_part(out_b, P, 1))

            # ---------------- per batch-tile pipeline ----------------
            for t in range(B_LOC // P):
                b0 = t * P
                h_tile = work.tile([P, H], f32, tag="h")
                nc.sync.dma_start(out=h_tile[:], in_=prev_h[b0:b0 + P, :])
                y_tile = work.tile([P, F], f32, tag="y")
                nc.sync.dma_start(out=y_tile[:], in_=y_in[b0:b0 + P, :])

                hT = work.tile([P, H], f32, tag="hT")
                for j in range(2):
                    pt = ps_tr.tile([P, P], f32)
                    nc.tensor.transpose(pt[:], h_tile[:, j * P:(j + 1) * P], ident[:])
                    nc.vector.tensor_copy(hT[:, j * P:(j + 1) * P], pt[:])
                yT1 = work.tile([F + 1, P], f32, tag="yT1")
                pt = ps_tr.tile([P, P], f32)
                nc.tensor.transpose(pt[0:F, :], y_tile[:], ident[:])
                nc.vector.tensor_copy(yT1[0:F, :], pt[0:F, :])
                nc.vector.memset(yT1[F:F + 1, :], 1.0)

                logits = ps_big.tile([P, S], f32, tag="big")
                nc.tensor.matmul(logits[:], hT[:, 0:P], awt0[:], start=True, stop=False)
                nc.tensor.matmul(logits[:], hT[:, P:H], awt1[:], start=False, stop=False)
                nc.tensor.matmul(
                    logits[:], yT1[0:F + 1, :], awt2[0:F + 1, :],
                    start=False, stop=True,
                )

                negmax = work.tile([P, 1], f32, tag="negmax")
                nc.vector.reduce_max(
                    negmax[:], logits[:], axis=mybir.AxisListType.X, negate=True
                )
                attn_e = work.tile([P, S], f32, tag="attn_e")
                sumexp = work.tile([P, 1], f32, tag="sumexp")
                nc.scalar.activation(
                    out=attn_e[:], in_=logits[:],
                    func=mybir.ActivationFunctionType.Exp,
                    bias=negmax[:], scale=1.0, accum_out=sumexp[:],
                )
                inv_sum = work.tile([P, 1], f32, tag="inv_sum")
                nc.vector.reciprocal(inv_sum[:], sumexp[:])

                # ctx accumulation: 512 diag matmuls, enc streamed in 2 MiB chunks
                ps_ctx = ps_ctxp.tile([P, H], f32, tag="ctx")
                for c in range(S // G):
                    enc_t = encp.tile([P, G, H], bf16, tag="enc")
                    # SWDGE casts fp32->bf16 inline; HBM reads stay fp32
                    nc.gpsimd.dma_start(
                        out=enc_t[:], in_=enc[b0:b0 + P, c * G:(c + 1) * G, :]
                    )
                    diag = diagp.tile([P, G, P], bf16, tag="diag")
                    nc.vector.tensor_tensor(
                        out=diag[:], in0=identG[:],
                        in1=_bcast_free(attn_e[:, c * G:(c + 1) * G], P),
                        op=mybir.AluOpType.mult,
                    )
                    for j in range(G):
                        s = c * G + j
                        nc.tensor.matmul(
                            ps_ctx[:], lhsT=diag[:, j, :], rhs=enc_t[:, j, :],
                            start=(s == 0), stop=(s == S - 1),
                        )

                ctx_sb = work.tile([P, H], f32, tag="ctx_sb")
                nc.vector.tensor_scalar_mul(ctx_sb[:], ps_ctx[:], inv_sum[:])

                ctxT = work.tile([P, H], f32, tag="ctxT")
                for j in range(2):
                    pt = ps_tr.tile([P, P], f32)
                    nc.tensor.transpose(pt[:], ctx_sb[:, j * P:(j + 1) * P], ident[:])
                    nc.vector.tensor_copy(ctxT[:, j * P:(j + 1) * P], pt[:])

                # gates r,z fused: sigmoid(ctx@Wi[rz] + h@Wh[rz] + bi[rz] + bh[rz])
                RZ = 2 * H
                ps_rz = ps_big.tile([P, RZ], f32, tag="big")
                nc.tensor.matmul(ps_rz[:], ctxT[:, 0:P], wit[0][:, 0:RZ], start=True, stop=False)
                nc.tensor.matmul(ps_rz[:], ctxT[:, P:H], wit[1][:, 0:RZ], start=False, stop=False)
                nc.tensor.matmul(ps_rz[:], hT[:, 0:P], wht[0][:, 0:RZ], start=False, stop=False)
                nc.tensor.matmul(ps_rz[:], hT[:, P:H], wht[1][:, 0:RZ], start=False, stop=False)
                nc.tensor.matmul(ps_rz[:], ones_row[:], bias_i[0:1, 0:RZ], start=False, stop=False)
                nc.tensor.matmul(ps_rz[:], ones_row[:], bias_h[0:1, 0:RZ], start=False, stop=True)

                ps_ni = ps_np.tile([P, H], f32, tag="n")
                nc.tensor.matmul(ps_ni[:], ctxT[:, 0:P], wit[0][:, RZ:3 * H], start=True, stop=False)
                nc.tensor.matmul(ps_ni[:], ctxT[:, P:H], wit[1][:, RZ:3 * H], start=False, stop=False)
                nc.tensor.matmul(ps_ni[:], ones_row[:], bias_i[0:1, RZ:3 * H], start=False, stop=True)
                ps_nh = ps_np.tile([P, H], f32, tag="n")
                nc.tensor.matmul(ps_nh[:], hT[:, 0:P], wht[0][:, RZ:3 * H], start=True, stop=False)
                nc.tensor.matmul(ps_nh[:], hT[:, P:H], wht[1][:, RZ:3 * H], start=False, stop=False)
                nc.tensor.matmul(ps_nh[:], ones_row[:], bias_h[0:1, RZ:3 * H], start=False, stop=True)

                rz_sb = work.tile([P, RZ], f32, tag="rz")
                nc.scalar.activation(
                    out=rz_sb[:], in_=ps_rz[:],
                    func=mybir.ActivationFunctionType.Sigmoid,
                )
                # n = tanh(i_n + r * h_n)
                rhn = work.tile([P, H], f32, tag="rhn")
                nc.vector.tensor_tensor(
                    out=rhn[:], in0=rz_sb[:, 0:H], in1=ps_nh[:],
                    op=mybir.AluOpType.mult,
                )
                pre_n = work.tile([P, H], f32, tag="pre_n")
                nc.vector.tensor_tensor(
                    out=pre_n[:], in0=rhn[:], in1=ps_ni[:], op=mybir.AluOpType.add
                )
                n_sb = work.tile([P, H], f32, tag="n_sb")
                nc.scalar.activation(
                    out=n_sb[:], in_=pre_n[:],
                    func=mybir.ActivationFunctionType.Tanh,
                )
                # h_new = n + z * (h - n)
                d_sb = work.tile([P, H], f32, tag="d_sb")
                nc.vector.tensor_tensor(
                    out=d_sb[:], in0=h_tile[:], in1=n_sb[:],
                    op=mybir.AluOpType.subtract,
                )
                zd = work.tile([P, H], f32, tag="zd")
                nc.vector.tensor_tensor(
                    out=zd[:], in0=rz_sb[:, H:RZ], in1=d_sb[:],
                    op=mybir.AluOpType.mult,
                )
                hnew_sb = work.tile([P, H], f32, tag="hnew")
                nc.vector.tensor_tensor(
                    out=hnew_sb[:], in0=zd[:], in1=n_sb[:], op=mybir.AluOpType.add
                )
                nc.sync.dma_start(out=hnew_o[b0:b0 + P, :], in_=hnew_sb[:])

                # out = h_new @ out_W^T + out_b
                # (tensor_tensor_reduce would fuse this, but its lowering
                # wedges the device on this runtime build — keep it split)
                prod = work.tile([P, H], f32, tag="prod")
                out_col = work.tile([P, 1], f32, tag="out_col")
                nc.vector.tensor_tensor(
                    out=prod[:], in0=hnew_sb[:], in1=outw_rep[:],
                    op=mybir.AluOpType.mult,
                )
                nc.vector.reduce_sum(
                    out_col[:], prod[:], axis=mybir.AxisListType.X
                )
                nc.vector.tensor_scalar_add(out_col[:], out_col[:], outb_col[:])
                nc.sync.dma_start(out=out_o[b0:b0 + P, :], in_=out_col[:])

    nc.compile()
    return nc


_NC_CACHE = None


def _get_nc():
    global _NC_CACHE
    if _NC_CACHE is None:
        _NC_CACHE = build_nc()
    return _NC_CACHE


def kernel(encoder_output, prev_hidden, y, attn_W, attn_b, W_ih, W_hh,
           b_ih, b_hh, out_W, out_b, **run_kwargs):
    from concourse.bass_utils import run_bass_kernel_spmd

    encoder_output = np.ascontiguousarray(np.asarray(encoder_output, np.float32))
    prev_hidden = np.ascontiguousarray(np.asarray(prev_hidden, np.float32))
    y = np.ascontiguousarray(np.asarray(y, np.float32))
    shared = {
        "attn_W": np.ascontiguousarray(np.asarray(attn_W, np.float32)),
        "attn_b": np.ascontiguousarray(np.asarray(attn_b, np.float32)),
        "W_ih": np.ascontiguousarray(np.asarray(W_ih, np.float32)),
        "W_hh": np.ascontiguousarray(np.asarray(W_hh, np.float32)),
        "b_ih": np.ascontiguousarray(np.asarray(b_ih, np.float32)),
        "b_hh": np.ascontiguousarray(np.asarray(b_hh, np.float32)),
        "out_W": np.ascontiguousarray(np.asarray(out_W, np.float32)),
        "out_b": np.ascontiguousarray(np.asarray(out_b, np.float32)),
    }
    in_maps = []
    for i in range(N_CORES):
        lo, hi = i * B_LOC, (i + 1) * B_LOC
        in_maps.append({
            "encoder_output": encoder_output[lo:hi],
            "prev_hidden": prev_hidden[lo:hi],
            "y": y[lo:hi],
            **shared,
        })

    nc = _get_nc()
    res = run_bass_kernel_spmd(nc, in_maps, core_ids=list(range(N_CORES)),
                               **run_kwargs)
    out = np.concatenate([res.results[i]["out"] for i in range(N_CORES)], axis=0)
    h_new = np.concatenate([res.results[i]["h_new"] for i in range(N_CORES)], axis=0)
    kernel.last_results = res
    return (out, h_new)


# revision 9
# speedup vs baseline: 1.4745x; 1.1031x over previous
"""AttentionDecoderCell (attention + GRUCell + linear head) on 8 trn2 cores.

Sharding: pure data parallel over batch B=2048 -> 8 cores x 256 rows.
Weights (attention / GRU / output linears) are replicated to every core.

Per-core kernel (B_loc=256 = 2 partition-tiles of 128):
  logits  = [prev_h | y | 1] @ [attn_W^T ; attn_b]        (PE, transposed on chip)
  attn_e  = exp(logits - rowmax), sumexp via ACT accum    (softmax, unnormalized)
  ctx     = (sum_s diag(attn_e[:, s]).T @ enc[:, s, :]) * 1/sumexp
            -- 512 accumulating PE matmuls per tile; diagonals built 16-at-a-
               time on DVE from a replicated-identity constant (broadcast AP)
  GRU     = torch GRUCell semantics, gates r/z fused in one [128,512] psum
  out     = h_new @ out_W^T + out_b  (single DVE tensor_tensor_reduce)

encoder_output ([B,S,H] f32, 1 GiB total) is the memory roofline; it streams
batch-major so every DMA moves 2 MiB with 16 KiB contiguous per partition.
"""

import os
import sys

import numpy as np

try:
    import concourse.bass as bass  # noqa: F401
except ImportError:  # pragma: no cover
    for _p in ("/opt/trn_rl_repo", os.path.expanduser("~/.axon_site/_ro/trn_rl_repo")):
        if os.path.isdir(_p) and _p not in sys.path:
            sys.path.insert(0, _p)
    import concourse.bass as bass  # noqa: F401

import concourse.bacc as bacc
import concourse.tile as tile
from concourse import mybir
from concourse.masks import make_identity

f32 = mybir.dt.float32
bf16 = mybir.dt.bfloat16

B, S, H, F = 2048, 512, 256, 64
N_CORES = 8
B_LOC = B // N_CORES          # 256 rows per core
P = 128                       # partition tile
G = 32                        # s-timesteps per diag build / enc DMA chunk
ENC_BUFS = 6                  # in-flight 4 MiB enc chunks


def _bcast_free(ap_obj, n):
    """Append a stride-0 free dim of size n to an AP (free-dim broadcast)."""
    return bass.AP(
        tensor=ap_obj.tensor,
        offset=ap_obj.offset,
        ap=list(ap_obj.ap) + [[0, n]],
    )


def _bcast_part(dram_tensor, p, free_elems):
    """Partition-broadcast a DRAM row across p partitions (stride-0)."""
    return bass.AP(tensor=dram_tensor, offset=0, ap=[[0, p], [1, free_elems]])


def build_nc():
    nc = bacc.Bacc(None, target_bir_lowering=False, debug=False)

    enc = nc.dram_tensor("encoder_output", [B_LOC, S, H], f32, kind="ExternalInput")
    prev_h = nc.dram_tensor("prev_hidden", [B_LOC, H], f32, kind="ExternalInput")
    y_in = nc.dram_tensor("y", [B_LOC, F], f32, kind="ExternalInput")
    attn_W = nc.dram_tensor("attn_W", [S, H + F], f32, kind="ExternalInput")
    attn_b = nc.dram_tensor("attn_b", [S], f32, kind="ExternalInput")
    W_ih = nc.dram_tensor("W_ih", [3 * H, H], f32, kind="ExternalInput")
    W_hh = nc.dram_tensor("W_hh", [3 * H, H], f32, kind="ExternalInput")
    b_ih = nc.dram_tensor("b_ih", [3 * H], f32, kind="ExternalInput")
    b_hh = nc.dram_tensor("b_hh", [3 * H], f32, kind="ExternalInput")
    out_W = nc.dram_tensor("out_W", [1, H], f32, kind="ExternalInput")
    out_b = nc.dram_tensor("out_b", [1], f32, kind="ExternalInput")
    out_o = nc.dram_tensor("out", [B_LOC, 1], f32, kind="ExternalOutput")
    hnew_o = nc.dram_tensor("h_new", [B_LOC, H], f32, kind="ExternalOutput")

    with tile.TileContext(nc) as tc:
        with (
            tc.tile_pool(name="consts", bufs=1) as consts,
            tc.tile_pool(name="wstage", bufs=1) as wstage,
            tc.tile_pool(name="work", bufs=2) as work,
            tc.tile_pool(name="encp", bufs=ENC_BUFS) as encp,
            tc.tile_pool(name="diagp", bufs=3) as diagp,
            tc.tile_pool(name="ps_big", bufs=2, space="PSUM") as ps_big,
            tc.tile_pool(name="ps_ctx", bufs=2, space="PSUM") as ps_ctxp,
            tc.tile_pool(name="ps_n", bufs=2, space="PSUM") as ps_np,
            tc.tile_pool(name="ps_tr", bufs=2, space="PSUM") as ps_tr,
        ):
            # ---------------- constants / weight preprocessing ----------------
            ident = consts.tile([P, P], f32)
            make_identity(nc, ident[:])
            identG = consts.tile([P, G, P], bf16)
            nc.gpsimd.memset(identG[:], 0.0)
            nc.gpsimd.affine_select(
                out=identG[:], in_=identG[:],
                compare_op=mybir.AluOpType.not_equal, fill=1.0, base=0,
                pattern=[[0, G], [-1, P]], channel_multiplier=1,
            )
            ones_row = consts.tile([1, P], f32)
            nc.vector.memset(ones_row[:], 1.0)

            # Stage each weight matrix with ONE DMA (small DMAs starve once the
            # enc stream saturates the SDMA engines), then transpose on PE.
            aw_st = wstage.tile([P, S // P, H + F], f32)
            nc.sync.dma_start(
                out=aw_st[:], in_=attn_W.rearrange("(r p) k -> p r k", p=P)
            )
            wi_st = wstage.tile([P, 3 * H // P, H], f32)
            nc.sync.dma_start(
                out=wi_st[:], in_=W_ih.rearrange("(r p) k -> p r k", p=P)
            )
            wh_st = wstage.tile([P, 3 * H // P, H], f32)
            nc.sync.dma_start(
                out=wh_st[:], in_=W_hh.rearrange("(r p) k -> p r k", p=P)
            )
            bias_i = consts.tile([1, 3 * H], f32)
            bias_h = consts.tile([1, 3 * H], f32)
            nc.sync.dma_start(out=bias_i[:], in_=b_ih.rearrange("(a g) -> a g", a=1))
            nc.sync.dma_start(out=bias_h[:], in_=b_hh.rearrange("(a g) -> a g", a=1))
            outw_rep = consts.tile([P, H], f32)
            nc.gpsimd.dma_start(out=outw_rep[:], in_=_bcast_part(out_W, P, H))
            outb_col = consts.tile([P, 1], f32)
            nc.gpsimd.dma_start(out=outb_col[:], in_=_bcast_part(out_b, P, 1))

            # attn_W^T as three k-chunk tiles (k = attn_in index; chunk2 also
            # carries attn_b in row 64 so the ones-column adds the bias)
            awt0 = consts.tile([P, S], f32)
            awt1 = consts.tile([P, S], f32)
            awt2 = consts.tile([P, S], f32)   # rows 0:64 = y part, row 64 = attn_b
            for i in range(S // P):
                for j, (dst, kw) in enumerate([(awt0, P), (awt1, P), (awt2, F)]):
                    pt = ps_tr.tile([P, P], f32)
                    nc.tensor.transpose(
                        pt[0:kw, :], aw_st[:, i, j * P:j * P + kw], ident[:]
                    )
                    nc.vector.tensor_copy(
                        dst[0:kw, i * P:(i + 1) * P], pt[0:kw, :]
                    )
            nc.sync.dma_start(
                out=awt2[F:F + 1, :], in_=attn_b.rearrange("(a s) -> a s", a=1)
            )


            # ---------------- phase A: attention prologue for both tiles ----------------
            tiles = []
            for t in range(B_LOC // P):
                b0 = t * P
                h_tile = work.tile([P, H], f32, tag="h")
                nc.sync.dma_start(out=h_tile[:], in_=prev_h[b0:b0 + P, :])
                y_tile = work.tile([P, F], f32, tag="y")
                nc.sync.dma_start(out=y_tile[:], in_=y_in[b0:b0 + P, :])

                hT = work.tile([P, H], f32, tag="hT")
                for j in range(2):
                    pt = ps_tr.tile([P, P], f32)
                    nc.tensor.transpose(pt[:], h_tile[:, j * P:(j + 1) * P], ident[:])
                    nc.vector.tensor_copy(hT[:, j * P:(j + 1) * P], pt[:])
                yT1 = work.tile([F + 1, P], f32, tag="yT1")
                pt = ps_tr.tile([P, P], f32)
                nc.tensor.transpose(pt[0:F, :], y_tile[:], ident[:])
                nc.vector.tensor_copy(yT1[0:F, :], pt[0:F, :])
                nc.vector.memset(yT1[F:F + 1, :], 1.0)

                logits = ps_big.tile([P, S], f32, tag="big")
                nc.tensor.matmul(logits[:], hT[:, 0:P], awt0[:], start=True, stop=False)
                nc.tensor.matmul(logits[:], hT[:, P:H], awt1[:], start=False, stop=False)
                nc.tensor.matmul(
                    logits[:], yT1[0:F + 1, :], awt2[0:F + 1, :],
                    start=False, stop=True,
                )

                negmax = work.tile([P, 1], f32, tag="negmax")
                nc.vector.reduce_max(
                    negmax[:], logits[:], axis=mybir.AxisListType.X, negate=True
                )
                attn_e = work.tile([P, S], f32, tag="attn_e")
                sumexp = work.tile([P, 1], f32, tag="sumexp")
                nc.scalar.activation(
                    out=attn_e[:], in_=logits[:],
                    func=mybir.ActivationFunctionType.Exp,
                    bias=negmax[:], scale=1.0, accum_out=sumexp[:],
                )
                inv_sum = work.tile([P, 1], f32, tag="inv_sum")
                nc.vector.reciprocal(inv_sum[:], sumexp[:])
                tiles.append((b0, h_tile, hT, attn_e, inv_sum))

            # GRU weight transposes (needed only by phase B tails)
            wit = [consts.tile([P, 3 * H], f32, name=f"wit{j}") for j in range(2)]
            wht = [consts.tile([P, 3 * H], f32, name=f"wht{j}") for j in range(2)]
            for st, dsts in ((wi_st, wit), (wh_st, wht)):
                for i in range(3 * H // P):
                    for j in range(2):
                        pt = ps_tr.tile([P, P], f32)
                        nc.tensor.transpose(pt[:], st[:, i, j * P:(j + 1) * P], ident[:])
                        nc.vector.tensor_copy(dsts[j][:, i * P:(i + 1) * P], pt[:])

            # ---------------- phase B: einsum + GRU per tile ----------------
            for t, (b0, h_tile, hT, attn_e, inv_sum) in enumerate(tiles):
                # ctx accumulation: 512 diag matmuls, enc streamed in 4 MiB chunks
                ps_ctx = ps_ctxp.tile([P, H], f32, tag="ctx")
                for c in range(S // G):
                    enc_t = encp.tile([P, G, H], bf16, tag="enc")
                    # SWDGE casts fp32->bf16 inline; HBM reads stay fp32
                    nc.gpsimd.dma_start(
                        out=enc_t[:], in_=enc[b0:b0 + P, c * G:(c + 1) * G, :]
                    )
                    diag = diagp.tile([P, G, P], bf16, tag="diag")
                    nc.vector.tensor_tensor(
                        out=diag[:], in0=identG[:],
                        in1=_bcast_free(attn_e[:, c * G:(c + 1) * G], P),
                        op=mybir.AluOpType.mult,
                    )
                    for j in range(G):
                        s = c * G + j
                        nc.tensor.matmul(
                            ps_ctx[:], lhsT=diag[:, j, :], rhs=enc_t[:, j, :],
                            start=(s == 0), stop=(s == S - 1),
                        )

                ctx_sb = work.tile([P, H], f32, tag="ctx_sb")
                nc.vector.tensor_scalar_mul(ctx_sb[:], ps_ctx[:], inv_sum[:])

                ctxT = work.tile([P, H], f32, tag="ctxT")
                for j in range(2):
                    pt = ps_tr.tile([P, P], f32)
                    nc.tensor.transpose(pt[:], ctx_sb[:, j * P:(j + 1) * P], ident[:])
                    nc.vector.tensor_copy(ctxT[:, j * P:(j + 1) * P], pt[:])

                # gates r,z fused: sigmoid(ctx@Wi[rz] + h@Wh[rz] + bi[rz] + bh[rz])
                RZ = 2 * H
                ps_rz = ps_big.tile([P, RZ], f32, tag="big")
                nc.tensor.matmul(ps_rz[:], ctxT[:, 0:P], wit[0][:, 0:RZ], start=True, stop=False)
                nc.tensor.matmul(ps_rz[:], ctxT[:, P:H], wit[1][:, 0:RZ], start=False, stop=False)
                nc.tensor.matmul(ps_rz[:], hT[:, 0:P], wht[0][:, 0:RZ], start=False, stop=False)
                nc.tensor.matmul(ps_rz[:], hT[:, P:H], wht[1][:, 0:RZ], start=False, stop=False)
                nc.tensor.matmul(ps_rz[:], ones_row[:], bias_i[0:1, 0:RZ], start=False, stop=False)
                nc.tensor.matmul(ps_rz[:], ones_row[:], bias_h[0:1, 0:RZ], start=False, stop=True)

                ps_ni = ps_np.tile([P, H], f32, tag="n")
                nc.tensor.matmul(ps_ni[:], ctxT[:, 0:P], wit[0][:, RZ:3 * H], start=True, stop=False)
                nc.tensor.matmul(ps_ni[:], ctxT[:, P:H], wit[1][:, RZ:3 * H], start=False, stop=False)
                nc.tensor.matmul(ps_ni[:], ones_row[:], bias_i[0:1, RZ:3 * H], start=False, stop=True)
                ps_nh = ps_np.tile([P, H], f32, tag="n")
                nc.tensor.matmul(ps_nh[:], hT[:, 0:P], wht[0][:, RZ:3 * H], start=True, stop=False)
                nc.tensor.matmul(ps_nh[:], hT[:, P:H], wht[1][:, RZ:3 * H], start=False, stop=False)
                nc.tensor.matmul(ps_nh[:], ones_row[:], bias_h[0:1, RZ:3 * H], start=False, stop=True)

                rz_sb = work.tile([P, RZ], f32, tag="rz")
                nc.scalar.activation(
                    out=rz_sb[:], in_=ps_rz[:],
                    func=mybir.ActivationFunctionType.Sigmoid,
                )
                # n = tanh(i_n + r * h_n)
                rhn = work.tile([P, H], f32, tag="rhn")
                nc.vector.tensor_tensor(
                    out=rhn[:], in0=rz_sb[:, 0:H], in1=ps_nh[:],
                    op=mybir.AluOpType.mult,
                )
                pre_n = work.tile([P, H], f32, tag="pre_n")
                nc.vector.tensor_tensor(
                    out=pre_n[:], in0=rhn[:], in1=ps_ni[:], op=mybir.AluOpType.add
                )
                n_sb = work.tile([P, H], f32, tag="n_sb")
                nc.scalar.activation(
                    out=n_sb[:], in_=pre_n[:],
                    func=mybir.ActivationFunctionType.Tanh,
                )
                # h_new = n + z * (h - n)
                d_sb = work.tile([P, H], f32, tag="d_sb")
                nc.vector.tensor_tensor(
                    out=d_sb[:], in0=h_tile[:], in1=n_sb[:],
                    op=mybir.AluOpType.subtract,
                )
                zd = work.tile([P, H], f32, tag="zd")
                nc.vector.tensor_tensor(
                    out=zd[:], in0=rz_sb[:, H:RZ], in1=d_sb[:],
                    op=mybir.AluOpType.mult,
                )
                hnew_sb = work.tile([P, H], f32, tag="hnew")
                nc.vector.tensor_tensor(
                    out=hnew_sb[:], in0=zd[:], in1=n_sb[:], op=mybir.AluOpType.add
                )
                nc.sync.dma_start(out=hnew_o[b0:b0 + P, :], in_=hnew_sb[:])

                # out = h_new @ out_W^T + out_b
                # (tensor_tensor_reduce would fuse this, but its lowering
                # wedges the device on this runtime build -- keep it split)
                prod = work.tile([P, H], f32, tag="prod")
                out_col = work.tile([P, 1], f32, tag="out_col")
                nc.vector.tensor_tensor(
                    out=prod[:], in0=hnew_sb[:], in1=outw_rep[:],
                    op=mybir.AluOpType.mult,
                )
                nc.vector.reduce_sum(
                    out_col[:], prod[:], axis=mybir.AxisListType.X
                )
                nc.vector.tensor_scalar_add(out_col[:], out_col[:], outb_col[:])
                nc.sync.dma_start(out=out_o[b0:b0 + P, :], in_=out_col[:])

    nc.compile()
    return nc


_NC_CACHE = None


def _get_nc():
    global _NC_CACHE
    if _NC_CACHE is None:
        _NC_CACHE = build_nc()
    return _NC_CACHE


def kernel(encoder_output, prev_hidden, y, attn_W, attn_b, W_ih, W_hh,
           b_ih, b_hh, out_W, out_b, **run_kwargs):
    from concourse.bass_utils import run_bass_kernel_spmd

    encoder_output = np.ascontiguousarray(np.asarray(encoder_output, np.float32))
    prev_hidden = np.ascontiguousarray(np.asarray(prev_hidden, np.float32))
    y = np.ascontiguousarray(np.asarray(y, np.float32))
    shared = {
        "attn_W": np.ascontiguousarray(np.asarray(attn_W, np.float32)),
        "attn_b": np.ascontiguousarray(np.asarray(attn_b, np.float32)),
        "W_ih": np.ascontiguousarray(np.asarray(W_ih, np.float32)),
        "W_hh": np.ascontiguousarray(np.asarray(W_hh, np.float32)),
        "b_ih": np.ascontiguousarray(np.asarray(b_ih, np.float32)),
        "b_hh": np.ascontiguousarray(np.asarray(b_hh, np.float32)),
        "out_W": np.ascontiguousarray(np.asarray(out_W, np.float32)),
        "out_b": np.ascontiguousarray(np.asarray(out_b, np.float32)),
    }
    in_maps = []
    for i in range(N_CORES):
        lo, hi = i * B_LOC, (i + 1) * B_LOC
        in_maps.append({
            "encoder_output": encoder_output[lo:hi],
            "prev_hidden": prev_hidden[lo:hi],
            "y": y[lo:hi],
            **shared,
        })

    nc = _get_nc()
    try:
        res = run_bass_kernel_spmd(nc, in_maps, core_ids=list(range(N_CORES)),
                                   **run_kwargs)
    except Exception:
        # A prior crashed process can leave the device in a transient
        # NRT_EXEC_UNIT_UNRECOVERABLE state; it clears after a reconnect.
        import time
        time.sleep(20)
        res = run_bass_kernel_spmd(nc, in_maps, core_ids=list(range(N_CORES)),
                                   **run_kwargs)
    out = np.concatenate([res.results[i]["out"] for i in range(N_CORES)], axis=0)
    h_new = np.concatenate([res.results[i]["h_new"] for i in range(N_CORES)], axis=0)
    kernel.last_results = res
    return (out, h_new)
